# revision 12
# baseline (speedup 1.0000x reference)
"""Causal self-attention (B=4, T=2048, D=1024, H=16) on 8 NeuronCores.

Sharding: core c handles batch b=c//2 and head-group hg=c%2 (8 of 16 heads).
Per core: column-parallel Wq/Wk/Wv (512 cols), row-parallel Wo (512 rows).
Host sums the two partial outputs per batch and adds bo. No collectives.

On-chip layout (all transposed; no on-chip transposes needed):
  xT [D=1024, T=2048] (host pre-transposed), qT/kT [512 dout, T],
  V natural [T, 8 heads x (64 dv + 1 ones col)].
  Scores computed as S^T [t_k, t_q] = kT.T @ qT; exp (no max subtraction --
  scores are O(4), fp32 exp safe); PV matmul out^T[dv, t_q] = V_aug.T @ expS^T
  with the ones column yielding sumexp for free; divide via reciprocal +
  K=1 broadcast matmul; final projection consumes out^T directly as lhsT.
All matmuls in float32r (1 cyc/row at N>=256).
"""

import os
from contextlib import ExitStack

import numpy as np

import concourse.bass as bass
import concourse.bacc as bacc
import concourse.mybir as mybir
import concourse.tile as tile
from concourse.bass_utils import run_bass_kernel_spmd

B, T, D, H, DK = 4, 2048, 1024, 16, 64
HL = 8  # heads per core
CD = HL * DK  # 512 local channels
NP = 128  # partitions
QB = 512  # query block / matmul moving dim
NDC = D // NP  # 8 din chunks
NTT = T // NP  # 16 t-tiles
NTB = T // QB  # 4 t-blocks
NPAIR = HL // 2  # 4 head pairs
F32 = mybir.dt.float32
F32R = mybir.dt.float32r
Exp = mybir.ActivationFunctionType.Exp
Identity = mybir.ActivationFunctionType.Identity

_CACHE: dict = {}


def _r(ap):
    return ap


def _build_nc():
    nc = bacc.Bacc("TRN2", target_bir_lowering=False, debug=False)
    xt = nc.dram_tensor("xt", [D, T], F32R, kind="ExternalInput")
    wq = nc.dram_tensor("wq", [D, CD], F32R, kind="ExternalInput")
    wk = nc.dram_tensor("wk", [D, CD], F32R, kind="ExternalInput")
    wv = nc.dram_tensor("wv", [D, CD], F32R, kind="ExternalInput")
    wo = nc.dram_tensor("wo", [CD, D], F32R, kind="ExternalInput")
    bqc = nc.dram_tensor("bqc", [NP, NPAIR], F32, kind="ExternalInput")
    bkc = nc.dram_tensor("bkc", [NP, NPAIR], F32, kind="ExternalInput")
    bvr = nc.dram_tensor("bvr", [1, CD], F32R, kind="ExternalInput")
    msk = nc.dram_tensor("msk", [4, NP, QB], F32, kind="ExternalInput")
    onesd = nc.dram_tensor("onesd", [NP, QB], F32R, kind="ExternalInput")
    y = nc.dram_tensor("y", [T, D], F32, kind="ExternalOutput")

    with tile.TileContext(nc) as tc, ExitStack() as ctx:
        _body(nc, tc, ctx, xt, wq, wk, wv, wo, bqc, bkc, bvr, msk, onesd, y)
    nc.compile()
    return nc


def _body(nc, tc, ctx, xt, wq, wk, wv, wo, bqc, bkc, bvr, msk, onesd, y):
    const = ctx.enter_context(tc.tile_pool(name="const", bufs=1))
    vpool = ctx.enter_context(tc.tile_pool(name="v", bufs=1))
    oatp = ctx.enter_context(tc.tile_pool(name="oat", bufs=1))
    xtp = ctx.enter_context(tc.tile_pool(name="xt", bufs=9))
    # PSUM: proj(2, shared w/ bcast) + score(2 tags x 2) + pv(2 tags x 1) = 8
    projps = ctx.enter_context(tc.tile_pool(name="projps", bufs=2, space="PSUM"))
    scoreps = ctx.enter_context(tc.tile_pool(name="scoreps", bufs=2, space="PSUM"))
    pvps = ctx.enter_context(tc.tile_pool(name="pvps", bufs=1, space="PSUM"))

    # constants
    ones_t = const.tile([1, QB], F32R)
    nc.sync.dma_start(ones_t[:], onesd[0:1, :])
    bq_sb = const.tile([NP, NPAIR], F32, tag="bq")
    nc.sync.dma_start(bq_sb[:], bqc[:])
    bk_sb = const.tile([NP, NPAIR], F32, tag="bk")
    nc.sync.dma_start(bk_sb[:], bkc[:])
    bv_sb = const.tile([1, CD], F32R, tag="bv")
    nc.sync.dma_start(bv_sb[:], bvr[:])
    msk_sb = const.tile([NP, 4, QB], F32, tag="msk")
    for o in range(4):
        nc.sync.dma_start(msk_sb[:, o, :], msk[o, :, :])
    # warm up the exp table set early (one-time ~2.7us load overlaps V phase)
    warm = const.tile([1, 2], F32, tag="warm")
    nc.vector.memset(warm[:], 0.0)
    nc.scalar.activation(warm[:], warm[:], Exp)

    # ---- V phase: V[t, dv] for all 8 heads, with ones column per head ----
    v_sb = [
        vpool.tile([NP, HL, DK + 1], F32R, tag=f"v{tt}", name=f"v{tt}")
        for tt in range(NTT)
    ]
    wvp_cm = tc.tile_pool(name="wvp", bufs=1)
    wvp = wvp_cm.__enter__()
    wv_sb = wvp.tile([NP, NDC, CD], F32R, tag="wv")
    for d in range(NDC):
        nc.sync.dma_start(wv_sb[:, d, :], wv[d * NP : (d + 1) * NP, :])
    for tb in range(NTB):
        xts = []
        for d in range(NDC):
            xtile = xtp.tile([NP, QB], F32R)
            nc.sync.dma_start(
                xtile[:], xt[d * NP : (d + 1) * NP, tb * QB : (tb + 1) * QB]
            )
            xts.append(xtile)
        for i in range(QB // NP):
            tt = tb * (QB // NP) + i
            ps = projps.tile([NP, CD], F32, tag="proj")
            for d in range(NDC):
                nc.tensor.matmul(
                    ps[:],
                    _r(xts[d][:, i * NP : (i + 1) * NP]),
                    _r(wv_sb[:, d, :]),
                    start=(d == 0),
                    stop=False,
                )
            # + ones_col x bv  (bias along free dim via K=1 rank-1 update)
            nc.tensor.matmul(
                ps[:], _r(ones_t[0:1, 0:NP]), _r(bv_sb[:]), start=False, stop=True
            )
            vt = v_sb[tt]
            nc.sync.dma_start(vt[:, :, DK : DK + 1], onesd[:, 0:HL])
            nc.vector.tensor_copy(vt[:, :, 0:DK], ps.rearrange("p (h k) -> p h k", h=HL))

    wvp_cm.__exit__(None, None, None)

    # ---- per head-pair: qT/kT projection then attention ----
    wqkp = ctx.enter_context(tc.tile_pool(name="wqk", bufs=2))
    qtp = ctx.enter_context(tc.tile_pool(name="qt", bufs=2))
    ktp = ctx.enter_context(tc.tile_pool(name="kt", bufs=2))
    expp = ctx.enter_context(tc.tile_pool(name="exp", bufs=2))
    smallp = ctx.enter_context(tc.tile_pool(name="small", bufs=2))
    oat = [oatp.tile([NP, T], F32R, tag=f"oat{c}", name=f"oat{c}") for c in range(NPAIR)]

    for c in range(NPAIR):
        wqc = wqkp.tile([NP, NDC, NP], F32R, tag="wqc")
        wkc = wqkp.tile([NP, NDC, NP], F32R, tag="wkc")
        for d in range(NDC):
            nc.sync.dma_start(
                wqc[:, d, :], wq[d * NP : (d + 1) * NP, c * NP : (c + 1) * NP]
            )
            nc.sync.dma_start(
                wkc[:, d, :], wk[d * NP : (d + 1) * NP, c * NP : (c + 1) * NP]
            )
        qt = qtp.tile([NP, T], F32R)
        kt_t = ktp.tile([NP, T], F32R)
        for tb in range(NTB):
            xts = []
            for d in range(NDC):
                xtile = xtp.tile([NP, QB], F32R)
                nc.sync.dma_start(
                    xtile[:], xt[d * NP : (d + 1) * NP, tb * QB : (tb + 1) * QB]
                )
                xts.append(xtile)
            psq = projps.tile([NP, QB], F32, tag="proj")
            for d in range(NDC):
                nc.tensor.matmul(
                    psq[:], _r(wqc[:, d, :]), _r(xts[d][:]),
                    start=(d == 0), stop=(d == NDC - 1),
                )
            nc.scalar.activation(
                qt[:, tb * QB : (tb + 1) * QB], psq[:], Identity,
                bias=bq_sb[:, c : c + 1],
            )
            psk = projps.tile([NP, QB], F32, tag="proj")
            for d in range(NDC):
                nc.tensor.matmul(
                    psk[:], _r(wkc[:, d, :]), _r(xts[d][:]),
                    start=(d == 0), stop=(d == NDC - 1),
                )
            nc.scalar.activation(
                kt_t[:, tb * QB : (tb + 1) * QB], psk[:], Identity,
                bias=bk_sb[:, c : c + 1],
            )

        for qb in range(NTB):
            nkt = 4 * qb + 4  # k-tiles 0..4qb+3 (last 4 are diagonal)
            pv = [
                pvps.tile([DK + 1, QB], F32, tag=f"pv{h}", name=f"pv{h}")
                for h in range(2)
            ]
            for kti in range(nkt):
                diag = kti >= 4 * qb
                sps = [
                    scoreps.tile([NP, QB], F32, tag=f"s{h}", name=f"s{h}")
                    for h in range(2)
                ]
                for h in range(2):
                    nc.tensor.matmul(
                        sps[h][:],
                        _r(kt_t[64 * h : 64 * h + 64, kti * NP : (kti + 1) * NP]),
                        _r(qt[64 * h : 64 * h + 64, qb * QB : (qb + 1) * QB]),
                        start=True, stop=True,
                        tile_position=(64 * h, 0),
                    )
                for h in range(2):
                    et = expp.tile([NP, QB], F32R, tag=f"e{h}")
                    if diag:
                        tmp = expp.tile([NP, QB], F32, tag="tmp")
                        nc.vector.tensor_add(
                            tmp[:], sps[h][:], msk_sb[:, kti - 4 * qb, :]
                        )
                        nc.scalar.activation(et[:], tmp[:], Exp, scale=0.125)
                    else:
                        nc.scalar.activation(et[:], sps[h][:], Exp, scale=0.125)
                    hh = 2 * c + h
                    nc.tensor.matmul(
                        pv[h][:],
                        _r(v_sb[kti][:, hh, :]),
                        _r(et[:]),
                        start=(kti == 0), stop=(kti == nkt - 1),
                    )
            for h in range(2):
                recip = smallp.tile([1, QB], F32R, tag="recip")
                with nc.allow_low_precision("fp32r softmax denom (2.4e-4)"):
                    nc.vector.reciprocal(recip[:], pv[h][DK : DK + 1, :])
                bc = projps.tile([NP, QB], F32, tag="proj")
                nc.tensor.matmul(
                    bc[0:DK, :], _r(ones_t[0:1, 0:DK]), _r(recip[:]),
                    start=True, stop=True,
                )
                bcs = smallp.tile([DK, QB], F32, tag="bcs")
                nc.vector.tensor_copy(bcs[:], bc[0:DK, :])
                nc.vector.tensor_mul(
                    oat[c][64 * h : 64 * h + 64, qb * QB : (qb + 1) * QB],
                    pv[h][0:DK, :],
                    bcs[:],
                )

    # ---- final projection: y[t, dout] = outAllT.T @ Wo ----
    wop = ctx.enter_context(tc.tile_pool(name="wop", bufs=1))
    wo_sb = wop.tile([NP, NDC // 2, D], F32R, tag="wo")
    for c in range(NPAIR):
        nc.sync.dma_start(wo_sb[:, c, :], wo[c * NP : (c + 1) * NP, :])
    for tt in range(NTT):
        for dh in range(2):
            ps = projps.tile([NP, QB], F32, tag="proj")
            for c in range(NPAIR):
                nc.tensor.matmul(
                    ps[:],
                    _r(oat[c][:, tt * NP : (tt + 1) * NP]),
                    _r(wo_sb[:, c, dh * QB : (dh + 1) * QB]),
                    start=(c == 0), stop=(c == NPAIR - 1),
                )
            ystage = smallp.tile([NP, QB], F32, tag="ystage", bufs=2)
            nc.vector.tensor_copy(ystage[:], ps[:])
            nc.sync.dma_start(
                y[tt * NP : (tt + 1) * NP, dh * QB : (dh + 1) * QB], ystage[:]
            )


def _install_ntff_hook_shim():
    """The agent image's antenv lacks axon_hooks, so trace=True under axon
    degrades. Provide the missing module and register the ctypes NTFF hook
    from trn_agent_boot. Best-effort: failures just mean no trace."""
    try:
        import sys
        import types

        if "antenv.axon_hooks" not in sys.modules:
            mod = types.ModuleType("antenv.axon_hooks")
            mod._hook = None
            mod.set_axon_ntff_profile_hook = lambda h: setattr(mod, "_hook", h)
            mod.get_axon_ntff_profile_hook = lambda: mod._hook
            sys.modules["antenv.axon_hooks"] = mod
            import antenv

            antenv.axon_hooks = mod
        from antenv.axon_hooks import (
            get_axon_ntff_profile_hook,
            set_axon_ntff_profile_hook,
        )

        if get_axon_ntff_profile_hook() is None:
            from trn_agent_boot.trn_boot import _ntff_profile_via_ctypes

            hook = _ntff_profile_via_ctypes("/opt/axon/libaxon_pjrt.so")
            if hook is not None:
                set_axon_ntff_profile_hook(hook)
    except Exception as e:  # noqa: BLE001
        print(f"ntff hook shim failed ({e}); running without trace")


def _round_f32r(a: np.ndarray) -> np.ndarray:
    """Round fp32 to fp32r (11-bit mantissa, low 12 bits zero), RNE."""
    u = np.ascontiguousarray(a, dtype=np.float32).view(np.uint32)
    u = (u + np.uint32(0x7FF) + ((u >> np.uint32(12)) & np.uint32(1))) & np.uint32(
        0xFFFFF000
    )
    return u.view(np.float32)


def _make_masks() -> np.ndarray:
    m = np.zeros((4, NP, QB), dtype=np.float32)
    kk = np.arange(NP)[:, None]
    qq = np.arange(QB)[None, :]
    for o in range(4):
        m[o] = np.where(qq >= kk + o * NP, 0.0, -1e30)
    return m


def kernel(x, Wq, bq, Wk, bk, Wv, bv, Wo, bo):
    x = np.ascontiguousarray(np.asarray(x, dtype=np.float32))
    Wq, bq = np.asarray(Wq, np.float32), np.asarray(bq, np.float32)
    Wk, bk = np.asarray(Wk, np.float32), np.asarray(bk, np.float32)
    Wv, bv = np.asarray(Wv, np.float32), np.asarray(bv, np.float32)
    Wo, bo = np.asarray(Wo, np.float32), np.asarray(bo, np.float32)

    if "nc" not in _CACHE:
        _CACHE["nc"] = _build_nc()
    nc = _CACHE["nc"]

    masks = _make_masks()
    in_maps = []
    for core in range(8):
        b, hg = core // 2, core % 2
        cs = slice(hg * CD, (hg + 1) * CD)
        in_maps.append(
            {
                "xt": _round_f32r(x[b].T),
                "wq": _round_f32r(Wq[:, cs]),
                "wk": _round_f32r(Wk[:, cs]),
                "wv": _round_f32r(Wv[:, cs]),
                "wo": _round_f32r(Wo[cs, :]),
                "bqc": np.ascontiguousarray(bq[cs].reshape(NPAIR, NP).T),
                "bkc": np.ascontiguousarray(bk[cs].reshape(NPAIR, NP).T),
                "bvr": _round_f32r(bv[cs].reshape(1, CD)),
                "msk": masks,
                "onesd": np.ones((NP, QB), dtype=np.float32),
            }
        )

    trace = bool(os.environ.get("KERNEL_TRACE"))
    if trace:
        _install_ntff_hook_shim()
    res = run_bass_kernel_spmd(
        nc, in_maps, core_ids=list(range(8)), trace=trace
    )
    _CACHE["last_results"] = res

    out = np.empty((B, T, D), dtype=np.float32)
    for b in range(B):
        out[b] = res.results[2 * b]["y"] + res.results[2 * b + 1]["y"] + bo
    return out
